# revision 1
# baseline (speedup 1.0000x reference)
"""DOFEN forward kernel for 8x Trainium2 NeuronCores (pure batch data-parallel).

Contract: kernel(**inputs) takes the FULL inputs from setup_inputs() and
returns the FULL [4096, 10] float32 output.

v2 design (per core, feature-partition layout [feat, batch], BC=512):
  P1:  O = tanh(0.5*U + bhat) via PE matmul + ACT; group sums s1 (PE,
       tile_position quadrants) and raw square-sums q1 = sum(O^2) (DVE
       squares + PE); q = max(q1 - 4*(s1/4)^2, 0) via DVE; no mean
       broadcast or centered-difference tiles needed.
  P1b: one fused Ln + Exp over all packs -> r1. A build-time reshape of
       the activation-table map keeps Ln/Exp/Relu/Copy/Square in a single
       table so the whole program needs only one table switch after tanh.
  P3:  conv2 on raw O with the group-mean correction folded in as a second
       chained matmul (m2corr x scp1); t = c2 * broadcast(r1) (the
       per-group scale commutes through the group-local conv); h =
       relu(t + b2) on DVE; GN2 stats the same sum/square-sum way; conv3
       on h with mean correction (scp2 * sw3col) on DVE; w = c3 * r2 with
       per-pack Ln/Exp; ew = exp(w + b3 - 4) staged and stored to DRAM in
       one DMA.  All engines balanced: PE ~90% busy through P3.
  MLP: per-forest row gather of ew (GF_CALL=4 keeps each gather at the
       1024-descriptor SWDGE ring limit); two matmuls per forest against
       the LN1-centered Ep@W1 fold (g2 packed 2-per-128-partitions with
       matching gather-pad offsets); relu split ACT/DVE; LN2 stats via
       one-hot accumulating matmuls; rstd2 batch chain; pass 2 scales z
       (DVE/Pool split, Pool via an ACT copy of the PSUM broadcast) into
       two alternating PSUM accumulators for fc2; rank-1 corrections;
       transpose out.

Hardware-legality notes learned the hard way: DVE tensor ops may read at
most ONE PSUM operand, Pool/GPSIMD must stay SBUF-only, and a single
dma_gather must not exceed 1024 descriptors.
"""

import os
import sys

for _p in ("/opt/trn_rl_repo", "/root/.axon_site/_ro/trn_rl_repo"):
    if os.path.isdir(_p) and _p not in sys.path:
        sys.path.insert(0, _p)

import numpy as np
import ml_dtypes

import concourse.bass as bass
import concourse.bacc as bacc
import concourse.tile as tile
from concourse import mybir
import concourse.bass_utils as bass_utils

# ---- problem shapes (hardcoded per contest contract) ----
B = 4096
NCOL = 100
NCOND = 64
D = 4
TOTAL = 6400           # n_col * n_cond
G = 1600               # n_rodt groups
NEST = 160
F = 100                # forests
H = 128                # hidden
C = 10                 # classes
EPS = 1e-5
NCORES = 8
BC = B // NCORES       # 512 per core
NT = TOTAL // 128      # 50 feature tiles
NPACK = (NT + 3) // 4  # 13 packed stats tiles (last covers 2 src tiles)
GPAD = NPACK * 128     # 1664 padded rodt rows
GF_CALL = int(os.environ.get("KGF", "4"))   # forests per dma_gather call
NCALLS = F // GF_CALL
LN2C = float(np.log(2.0))

f32 = mybir.dt.float32
bf16 = mybir.dt.float16   # 16-bit activations/weights use fp16 (11-bit mantissa)
fp8 = mybir.dt.float8e4
i16 = mybir.dt.int16
AF = mybir.ActivationFunctionType
OP = mybir.AluOpType
PM = mybir.MatmulPerfMode

BF = np.float16
F8 = ml_dtypes.float8_e4m3


def _host_prep(inputs):
    """Fold all parameter algebra on the host; returns dict of device arrays."""
    f64 = np.float64
    x = np.asarray(inputs["x"], np.float32)
    w1 = np.asarray(inputs["w1"], f64)
    b1 = np.asarray(inputs["b1"], f64)
    perm = np.asarray(inputs["perm"], np.int64)
    gn1_w = np.asarray(inputs["gn1_w"], f64)
    gn1_b = np.asarray(inputs["gn1_b"], f64)
    conv2_w = np.asarray(inputs["conv2_w"], f64)
    conv2_b = np.asarray(inputs["conv2_b"], f64)
    gn2_w = np.asarray(inputs["gn2_w"], f64)
    gn2_b = np.asarray(inputs["gn2_b"], f64)
    conv3_w = np.asarray(inputs["conv3_w"], f64)
    conv3_b = np.asarray(inputs["conv3_b"], f64)
    swr = np.asarray(inputs["swr"], np.int64)
    E = np.asarray(inputs["E"], f64)
    ln1_w = np.asarray(inputs["ln1_w"], f64)
    ln1_b = np.asarray(inputs["ln1_b"], f64)
    fc1_w = np.asarray(inputs["fc1_w"], f64)
    fc1_b = np.asarray(inputs["fc1_b"], f64)
    ln2_w = np.asarray(inputs["ln2_w"], f64)
    ln2_b = np.asarray(inputs["ln2_b"], f64)
    fc2_w = np.asarray(inputs["fc2_w"], f64)
    fc2_b = np.asarray(inputs["fc2_b"], f64)

    dev = {}

    # --- O-matmul weights: feature t corresponds to reference feature perm[t]
    p = perm
    c_arr = p // NCOL
    j_arr = p % NCOL
    What = np.zeros((100, TOTAL), np.float32)
    What[j_arr, np.arange(TOTAL)] = w1[j_arr, c_arr]
    dev["what"] = What.astype(BF)
    # folded into the tanh bias: tanh(0.5*U + 0.5*bhat)
    dev["bhat"] = (0.5 * b1[j_arr, c_arr]).reshape(NT, 128).T.astype(np.float32).copy()

    # --- GN1 affine folded into conv2 weights/bias ---
    w2f = conv2_w * gn1_w.reshape(G, D, 1)                 # [G, D, D]
    b2f = conv2_b.reshape(G, D) + np.einsum("gi,gio->go", gn1_b.reshape(G, D), conv2_w)
    bd2 = np.zeros((128, NT * 128), np.float32)
    gi = np.arange(32)
    for t in range(NT):
        g0 = 32 * t
        for i in range(D):
            for o in range(D):
                bd2[4 * gi + i, t * 128 + 4 * gi + o] = w2f[g0 + gi, i, o]
    dev["bd2"] = bd2.astype(BF)

    # conv2 group-mean correction: chained matmul  c2 -= m2corr^T scp1,
    # scp1[32j+u] = mean of group 32t+u (j = t%4).  m2corr[32j+u, t*128+p]
    # = -sum_i w2f[32t+u, i, p%4]  iff p//4 == u.
    m2corr = np.zeros((128, NT * 128), np.float32)
    w2s = w2f.sum(1)                                       # [G, D] sum over i
    for t in range(NT):
        j = t % 4
        pp = np.arange(128)
        m2corr[32 * j + pp // 4, t * 128 + pp] = -w2s[32 * t + pp // 4, pp % 4]
    dev["m2corr"] = m2corr.astype(BF)

    # --- GN2 + conv3 ---
    w3f = conv3_w[:, :, 0] * gn2_w.reshape(G, D)
    b3f = conv3_b + (gn2_b.reshape(G, D) * conv3_w[:, :, 0]).sum(1)
    bd3 = np.zeros((128, NT * 32), np.float32)
    for t in range(NT):
        g0 = 32 * t
        for i in range(D):
            bd3[4 * gi + i, t * 32 + gi] = w3f[g0 + gi, i]
    dev["bd3"] = bd3.astype(BF)

    # conv3 mean correction scalars: u = scp2 * sw3col, then c3 -= I u
    sw3 = w3f.sum(1)                                       # [G]
    sw3_pad = np.zeros(GPAD, np.float64)
    sw3_pad[:G] = sw3
    dev["sw3col"] = sw3_pad.reshape(NPACK, 128).T.astype(np.float32).copy()

    # ones-block for group sums: [128, 32]
    onesblk = np.zeros((128, 32), np.float32)
    onesblk[np.arange(128), np.arange(128) // 4] = 1.0
    dev["onesblk"] = onesblk.astype(BF)

    # group-stat broadcast selectors: bcastj[k, j*128+m] = (k == 32*j + m//4)
    bcastj = np.zeros((128, 4 * 128), np.float32)
    for j in range(4):
        m = np.arange(128)
        bcastj[32 * j + m // 4, j * 128 + m] = 1.0
    dev["bcastj"] = bcastj.astype(BF)

    # identity-style lhsT for chained PSUM fixups
    dev["negi"] = (-np.eye(128, dtype=np.float32)).astype(BF)
    dev["neg4i"] = (-4.0 * np.eye(128, dtype=np.float32)).astype(BF)

    # bias columns
    b2f_col = np.zeros((128, NT), np.float32)
    pp = np.arange(128)
    for t in range(NT):
        b2f_col[pp, t] = b2f[32 * t + pp // 4, pp % 4]
    dev["b2fcol"] = b2f_col
    b3f_pad = np.zeros(GPAD, np.float64)
    b3f_pad[:G] = b3f
    dev["b3fcol"] = (b3f_pad - 4.0).reshape(NPACK, 128).T.astype(np.float32).copy()

    # --- MLP folds ---
    W1p = ln1_w[:, None] * fc1_w                           # [128, 128]
    b1p = fc1_b + ln1_b @ fc1_w
    if np.abs(b1p).max() > 1e-12:
        raise NotImplementedError(
            "fused kernel assumes fc1_b + ln1_b@fc1_w == 0 (true for this problem)"
        )
    Ep = E[swr]                                            # [F, 160, 128]
    muEp = Ep.mean(2)                                      # [F, 160]
    cs1 = W1p.sum(0)                                       # [128]
    Gall = np.einsum("fed,dh->feh", Ep, W1p) - muEp[:, :, None] * cs1[None, None, :]
    g1 = np.ascontiguousarray(Gall[:, :128, :].transpose(1, 0, 2)).reshape(128, F * H)
    dev["g1"] = g1.astype(BF)
    g2p = np.zeros((128, (F // 2) * H), np.float32)
    for f in range(F):
        a, f2 = f % 2, f // 2
        g2p[64 * a:64 * a + 32, f2 * H:(f2 + 1) * H] = Gall[f, 128:, :]
    dev["g2"] = g2p.astype(BF)

    W2p = ln2_w[:, None] * fc2_w                           # [128, 10]
    b2p = fc2_b + ln2_b @ fc2_w                            # [10]
    W2pp = np.sqrt(128.0) * W2p
    cs2 = W2pp.sum(0)
    dev["w2pp"] = W2pp.astype(BF)
    dev["negcs2"] = (-cs2).reshape(1, C).astype(BF)
    dev["b2ppx"] = (float(F) * b2p).reshape(1, C).astype(BF)

    # one-hot columns for LN2 stats accumulation (two 50-row halves)
    FH = F // 2
    ohcol = np.zeros((128, F * FH), np.float32)
    for f in range(F):
        ohcol[:, f * FH + (f % FH)] = 1.0
    dev["ohcol"] = ohcol.astype(BF)

    # selector rows for rstd2 broadcast: selq[k, f*128+m] = (k == f%50)
    FH2 = F // 2
    selq = np.zeros((FH2, F * 128), np.float32)
    for f in range(F):
        selq[f % FH2, f * 128:(f + 1) * 128] = 1.0
    dev["selq"] = selq.astype(BF)

    # gather indices, wrapped in 16 partitions per call of GF_CALL forests
    idx_cols = []
    for call in range(NCALLS):
        L = []
        for f in range(call * GF_CALL, (call + 1) * GF_CALL):
            a = f % 2
            L.extend(swr[f, :128].tolist())
            L.extend([0] * (64 * a))
            L.extend(swr[f, 128:160].tolist())
            L.extend([0] * (96 - 64 * a))
        L = np.asarray(L, np.int16)
        wrap = L.reshape(-1, 16).T
        idx_cols.append(np.tile(wrap, (8, 1)))
    dev["gidx"] = np.concatenate(idx_cols, axis=1)  # [128, F*16] int16

    return dev, x


def _patched_act_tables(orig_fn):
    """Strip the shared funcs from all but one superset table so the
    act-table-load pass keeps a single table for everything after tanh."""
    def wrapper(arch):
        tabs = {k: set(v) for k, v in orig_fn(arch).items()}
        need = {AF.Ln, AF.Exp, AF.Relu, AF.Copy, AF.Square}
        s6 = None
        for name, s in tabs.items():
            if need <= s and AF.Tanh not in s:
                s6 = name
                break
        if s6 is None:
            return tabs
        for name in tabs:
            if name != s6:
                tabs[name] = tabs[name] - need
        return tabs
    return wrapper


def _build_program():
    """Build the per-core Bass program."""
    if os.environ.get("KNOACTPATCH", "0") == "1":
        return _build_program_inner()
    _orig_gat = bacc.get_activation_tables
    bacc.get_activation_tables = _patched_act_tables(_orig_gat)
    try:
        return _build_program_inner()
    finally:
        bacc.get_activation_tables = _orig_gat


def _build_program_inner():
    nc = bacc.Bacc("TRN2", debug=False, num_devices=NCORES)

    def din(name, shape, dt):
        return nc.dram_tensor(name, list(shape), dt, kind="ExternalInput").ap()

    x_d = din("x_shard", [BC, NCOL], f32)
    what_d = din("what", [100, TOTAL], bf16)
    bhat_d = din("bhat", [128, NT], f32)
    bd2_d = din("bd2", [128, NT * 128], bf16)
    m2corr_d = din("m2corr", [128, NT * 128], bf16)
    bd3_d = din("bd3", [128, NT * 32], bf16)
    sw3col_d = din("sw3col", [128, NPACK], f32)
    onesblk_d = din("onesblk", [128, 32], bf16)
    bcastj_d = din("bcastj", [128, 4 * 128], bf16)
    negi_d = din("negi", [128, 128], bf16)
    neg4i_d = din("neg4i", [128, 128], bf16)
    b2fcol_d = din("b2fcol", [128, NT], f32)
    b3fcol_d = din("b3fcol", [128, NPACK], f32)
    g1_d = din("g1", [128, F * H], bf16)
    g2_d = din("g2", [128, (F // 2) * H], bf16)
    w2pp_d = din("w2pp", [H, C], bf16)
    negcs2_d = din("negcs2", [1, C], bf16)
    b2ppx_d = din("b2ppx", [1, C], bf16)
    selq_d = din("selq", [F // 2, F * 128], bf16)
    ohcol_d = din("ohcol", [128, (F // 2) * F], bf16)
    gidx_d = din("gidx", [128, F * 16], i16)

    y_d = nc.dram_tensor("y_out", [BC, C], f32, kind="ExternalOutput").ap()

    from contextlib import ExitStack
    from concourse.masks import make_identity

    with tile.TileContext(nc) as tc, ExitStack() as ctx:
        persist = ctx.enter_context(tc.tile_pool(name="persist", bufs=1))
        dram_pool = ctx.enter_context(tc.tile_pool(name="drams", bufs=1, space="DRAM"))

        ident = persist.tile([128, 128], f32)
        make_identity(nc, ident[:])
        onesrow = persist.tile([1, BC], bf16)
        nc.vector.memset(onesrow[:], 1.0)
        eps4 = persist.tile([128, 1], f32)
        nc.vector.memset(eps4[:], 4.0 * EPS)
        epsH = persist.tile([128, 1], f32)
        nc.vector.memset(epsH[:], float(H) * EPS)
        ln2b = persist.tile([128, 1], f32)
        nc.vector.memset(ln2b[:], LN2C)

        # MLP consts: tiles allocated up front, DMAs issued later so the
        # phi2-critical loads (x, what) go first in the queue
        gidx = persist.tile([128, F * 16], i16)
        ohcol_sb = persist.tile([128, F * (F // 2)], bf16)
        w2pp = persist.tile([H, C], bf16)
        negcs2 = persist.tile([1, C], bf16)
        b2ppx = persist.tile([1, C], bf16)
        g1_sb = persist.tile([128, F * H], bf16)
        g2_sb = persist.tile([128, (F // 2) * H], bf16)

        ew_dram = dram_pool.tile([GPAD, BC], bf16)

        # ---------- phi2 ----------
        with ExitStack() as phi_ctx:
            pconst = phi_ctx.enter_context(tc.tile_pool(name="pconst", bufs=1))
            onesblk = pconst.tile([128, 32], bf16)
            nc.sync.dma_start(out=onesblk[:], in_=onesblk_d)
            bcastj = pconst.tile([128, 4 * 128], bf16)
            nc.sync.dma_start(out=bcastj[:], in_=bcastj_d)
            negi = pconst.tile([128, 128], bf16)
            nc.sync.dma_start(out=negi[:], in_=negi_d)
            neg4i = pconst.tile([128, 128], bf16)
            nc.sync.dma_start(out=neg4i[:], in_=neg4i_d)
            b2fcol = pconst.tile([128, NT], f32)
            nc.sync.dma_start(out=b2fcol[:], in_=b2fcol_d)
            b3fcol = pconst.tile([128, NPACK], f32)
            nc.sync.dma_start(out=b3fcol[:], in_=b3fcol_d)
            sw3col = pconst.tile([128, NPACK], f32)
            nc.sync.dma_start(out=sw3col[:], in_=sw3col_d)
            bd2_sb = pconst.tile([128, NT * 128], bf16)

            ofall = pconst.tile([128, NT, BC], bf16)
            scp1_all = pconst.tile([128, NPACK, BC], bf16)
            r1_all = pconst.tile([128, NPACK, BC], bf16)
            qall = pconst.tile([128, NPACK, BC], bf16)
            nc.vector.memset(qall[64:128, NPACK - 1, :], 1.0)

            # ---- P1 (ACT set: tanh/copy): O, group sums, square sums
            with ExitStack() as sA:
                pA = sA.enter_context(tc.tile_pool(name="pA", bufs=1))
                xts = []
                for bt in range(BC // 128):
                    x_t = pA.tile([128, NCOL], f32, tag="xload", name=f"xl{bt}")
                    nc.sync.dma_start(out=x_t[:], in_=x_d[bt * 128:(bt + 1) * 128, :])
                    xts.append(x_t)
                what_sb = pA.tile([100, TOTAL], bf16)
                nc.sync.dma_start(out=what_sb[:], in_=what_d)
                bhat_sb = pA.tile([128, NT], f32)
                nc.sync.dma_start(out=bhat_sb[:], in_=bhat_d)
                # deferred heavy/late consts, behind the phi2-critical loads
                nc.sync.dma_start(out=bd2_sb[:], in_=bd2_d)
                nc.sync.dma_start(out=gidx[:], in_=gidx_d)
                nc.sync.dma_start(out=ohcol_sb[:], in_=ohcol_d)
                nc.sync.dma_start(out=w2pp[:], in_=w2pp_d)
                nc.sync.dma_start(out=negcs2[:], in_=negcs2_d)
                nc.sync.dma_start(out=b2ppx[:], in_=b2ppx_d)

                x_aug = pA.tile([100, BC], bf16)
                with tc.tile_pool(name="xtp", bufs=2, space="PSUM") as xtpool:
                    for bt in range(BC // 128):
                        x_ps = xtpool.tile([NCOL, 128], f32, tag="xtps")
                        nc.tensor.transpose(out=x_ps[:], in_=xts[bt][:], identity=ident[:])
                        nc.vector.tensor_copy(
                            out=x_aug[0:NCOL, bt * 128:(bt + 1) * 128], in_=x_ps[:]
                        )

                popool = sA.enter_context(tc.tile_pool(name="popool", bufs=2, space="PSUM"))
                st1 = sA.enter_context(tc.tile_pool(name="st1", bufs=2, space="PSUM"))
                osqp = sA.enter_context(tc.tile_pool(name="osqp", bufs=3))
                sq1p = sA.enter_context(tc.tile_pool(name="sq1p", bufs=2))

                for st in range(NPACK):
                    tiles = list(range(4 * st, min(4 * st + 4, NT)))
                    nrow = 32 * len(tiles)
                    s1P = st1.tile([128, BC], f32, tag="s1")
                    q1P = st1.tile([128, BC], f32, tag="q1")
                    for t in tiles:
                        j = t % 4
                        po = popool.tile([128, BC], f32, tag="po")
                        nc.tensor.matmul(
                            out=po[:], lhsT=what_sb[:, t * 128:(t + 1) * 128],
                            rhs=x_aug[:], start=True, stop=True,
                        )
                        o_t = ofall[:, t, :]
                        nc.scalar.activation(
                            out=o_t, in_=po[:], func=AF.Tanh, scale=0.5,
                            bias=bhat_sb[:, t:t + 1],
                        )
                        nc.tensor.matmul(
                            out=s1P[32 * j:32 * j + 32, :], lhsT=onesblk[:],
                            rhs=o_t, start=True, stop=True,
                            tile_position=(0, 32 * j),
                        )
                        osq = osqp.tile([128, BC], bf16, tag="osq")
                        if t % 5 < 2:
                            nc.gpsimd.tensor_tensor(out=osq[:], in0=o_t, in1=o_t, op=OP.mult)
                        else:
                            nc.vector.tensor_tensor(out=osq[:], in0=o_t, in1=o_t, op=OP.mult)
                        nc.tensor.matmul(
                            out=q1P[32 * j:32 * j + 32, :], lhsT=onesblk[:],
                            rhs=osq[:], start=True, stop=True,
                            tile_position=(0, 32 * j),
                        )
                    # scp1 = s/4 (group mean); q -= 4*scp1^2
                    nc.scalar.activation(
                        out=scp1_all[:nrow, st, :], in_=s1P[:nrow], func=AF.Copy,
                        scale=0.25,
                    )
                    s1sq = sq1p.tile([128, BC], bf16, tag="s1sq")
                    nc.vector.tensor_tensor(
                        out=s1sq[:nrow], in0=scp1_all[:nrow, st, :],
                        in1=scp1_all[:nrow, st, :], op=OP.mult,
                    )
                    s1sqm = sq1p.tile([128, BC], bf16, tag="s1sqm")
                    nc.vector.tensor_scalar_mul(
                        out=s1sqm[:nrow], in0=s1sq[:nrow], scalar1=-4.0
                    )
                    qfix = sq1p.tile([128, BC], bf16, tag="qfix")
                    nc.vector.tensor_tensor(
                        out=qfix[:nrow], in0=s1sqm[:nrow], in1=q1P[:nrow], op=OP.add
                    )
                    # clamp >= 0 (f16 rounding can push sum(x^2) - s^2/4
                    # slightly negative -> Ln NaN)
                    nc.vector.tensor_scalar_max(
                        out=qall[:nrow, st, :], in0=qfix[:nrow], scalar1=0.0
                    )

            # conv2-correction + conv3 weights: loaded while P1b's ln/exp run
            pBconst = phi_ctx.enter_context(tc.tile_pool(name="pBconst", bufs=1))
            m2corr_sb = pBconst.tile([128, NT * 128], bf16)
            nc.sync.dma_start(out=m2corr_sb[:], in_=m2corr_d)
            bd3_sb = pBconst.tile([128, NT * 32], bf16)
            nc.sync.dma_start(out=bd3_sb[:], in_=bd3_d)

            # ---- P1b: GN1 rstd -- one fused Ln, one fused Exp (keeping them
            # unsplit avoids scheduler-interleaved act-table switches)
            nc.scalar.activation(out=qall[:], in_=qall[:], func=AF.Ln, bias=eps4[:])
            nc.scalar.activation(
                out=r1_all[:], in_=qall[:], func=AF.Exp, scale=-0.5, bias=ln2b[:],
            )

            # ---- P3a: conv2 + relu + GN2 stats + conv3 raw (no Ln -> one table)
            with ExitStack() as sB:
                c2pool = sB.enter_context(tc.tile_pool(name="c2pool", bufs=2, space="PSUM"))
                rbpool = sB.enter_context(tc.tile_pool(name="rbpool", bufs=2, space="PSUM"))
                st2 = sB.enter_context(tc.tile_pool(name="st2", bufs=1, space="PSUM"))
                c3pool = sB.enter_context(tc.tile_pool(name="c3pool", bufs=2, space="PSUM"))
                tpool = sB.enter_context(tc.tile_pool(name="tpool", bufs=2))
                hpool = sB.enter_context(tc.tile_pool(name="hpool", bufs=1))
                hsqp = sB.enter_context(tc.tile_pool(name="hsqp", bufs=2))
                packp = sB.enter_context(tc.tile_pool(name="packp", bufs=1))
                ewkeep = sB.enter_context(tc.tile_pool(name="ewkeep", bufs=1))
                ew_sb = ewkeep.tile([128, NPACK, BC], bf16)
                nc.vector.memset(ew_sb[64:128, NPACK - 1, :], 0.0)

                for st in range(NPACK):
                    tiles = list(range(4 * st, min(4 * st + 4, NT)))
                    nrow = 32 * len(tiles)
                    s2P = st2.tile([128, BC], f32, tag="s2")
                    q2P = st2.tile([128, BC], f32, tag="q2")
                    c3P = c3pool.tile([128, BC], f32, tag="c3")
                    for t in tiles:
                        j = t % 4
                        c2p = c2pool.tile([128, BC], f32, tag="c2")
                        nc.tensor.matmul(
                            out=c2p[:], lhsT=bd2_sb[:, t * 128:(t + 1) * 128],
                            rhs=ofall[:, t, :], start=True, stop=False,
                        )
                        nc.tensor.matmul(
                            out=c2p[:], lhsT=m2corr_sb[0:nrow, t * 128:(t + 1) * 128],
                            rhs=scp1_all[0:nrow, st, :], start=False, stop=True,
                        )
                        rbp = rbpool.tile([128, BC], f32, tag="rb")
                        nc.tensor.matmul(
                            out=rbp[:], lhsT=bcastj[0:nrow, j * 128:(j + 1) * 128],
                            rhs=r1_all[0:nrow, st, :], start=True, stop=True,
                        )
                        c2s = tpool.tile([128, BC], bf16, tag="c2s")
                        nc.scalar.activation(out=c2s[:], in_=c2p[:], func=AF.Copy)
                        tsb = tpool.tile([128, BC], bf16, tag="tsb")
                        nc.vector.tensor_tensor(
                            out=tsb[:], in0=c2s[:], in1=rbp[:], op=OP.mult
                        )
                        # h = relu(t + b2) on DVE (4x tensor_scalar)
                        hf = hpool.tile([128, BC], bf16, tag=f"h{j}")
                        nc.vector.tensor_scalar(
                            out=hf[:], in0=tsb[:], scalar1=b2fcol[:, t:t + 1],
                            scalar2=0.0, op0=OP.add, op1=OP.max,
                        )
                        nc.tensor.matmul(
                            out=s2P[32 * j:32 * j + 32, :], lhsT=onesblk[:],
                            rhs=hf[:], start=True, stop=True,
                            tile_position=(0, 32 * j),
                        )
                        hsq = hsqp.tile([128, BC], bf16, tag="hsq")
                        nc.gpsimd.tensor_tensor(out=hsq[:], in0=hf[:], in1=hf[:], op=OP.mult)
                        nc.tensor.matmul(
                            out=q2P[32 * j:32 * j + 32, :], lhsT=onesblk[:],
                            rhs=hsq[:], start=True, stop=True,
                            tile_position=(0, 32 * j),
                        )
                        nc.tensor.matmul(
                            out=c3P[32 * j:32 * j + 32, :],
                            lhsT=bd3_sb[:, t * 32:(t + 1) * 32],
                            rhs=hf[:], start=True, stop=True,
                            tile_position=(0, 32 * j),
                        )
                    # pack tail: GN2 rstd + conv3 corrections + exp, per pack
                    scp2 = packp.tile([128, BC], bf16, tag="scp2")
                    nc.scalar.activation(
                        out=scp2[:nrow], in_=s2P[:nrow], func=AF.Copy, scale=0.25,
                    )
                    s2sq = packp.tile([128, BC], bf16, tag="s2sq")
                    nc.vector.tensor_tensor(
                        out=s2sq[:nrow], in0=scp2[:nrow], in1=scp2[:nrow], op=OP.mult
                    )
                    s2sqm = packp.tile([128, BC], bf16, tag="s2sqm")
                    nc.vector.tensor_scalar_mul(
                        out=s2sqm[:nrow], in0=s2sq[:nrow], scalar1=-4.0
                    )
                    q2fix = packp.tile([128, BC], bf16, tag="q2fix")
                    nc.vector.tensor_tensor(
                        out=q2fix[:nrow], in0=s2sqm[:nrow], in1=q2P[:nrow], op=OP.add
                    )
                    nc.vector.tensor_scalar_max(
                        out=q2fix[:nrow], in0=q2fix[:nrow], scalar1=0.0
                    )
                    qln2 = packp.tile([128, BC], f32, tag="qln2")
                    nc.scalar.activation(
                        out=qln2[:nrow], in_=q2fix[:nrow], func=AF.Ln, bias=eps4[:nrow],
                    )
                    r2p = packp.tile([128, BC], bf16, tag="r2p")
                    nc.scalar.activation(
                        out=r2p[:nrow], in_=qln2[:nrow], func=AF.Exp, scale=-0.5,
                        bias=ln2b[:nrow],
                    )
                    u = packp.tile([128, BC], bf16, tag="u")
                    nc.vector.tensor_scalar(
                        out=u[:nrow], in0=scp2[:nrow], scalar1=sw3col[0:nrow, st:st + 1],
                        scalar2=None, op0=OP.mult,
                    )
                    v3 = packp.tile([128, BC], bf16, tag="v3")
                    nc.vector.tensor_tensor(
                        out=v3[:nrow], in0=c3P[:nrow], in1=u[:nrow], op=OP.subtract
                    )
                    wsb = packp.tile([128, BC], bf16, tag="wsb")
                    nc.vector.tensor_tensor(
                        out=wsb[:nrow], in0=v3[:nrow], in1=r2p[:nrow], op=OP.mult
                    )
                    nc.scalar.activation(
                        out=ew_sb[:nrow, st, :], in_=wsb[:nrow], func=AF.Exp,
                        bias=b3fcol[:nrow, st:st + 1],
                    )
                nc.sync.dma_start(
                    out=ew_dram[:].rearrange("(c p) w -> p c w", p=128), in_=ew_sb[:]
                )
            # heavy MLP consts queued behind the ew store; chunked so the
            # first forests' weights arrive before the first z matmuls
            for ck in range(4):
                cw = F * H // 4
                nc.sync.dma_start(
                    out=g1_sb[:, ck * cw:(ck + 1) * cw], in_=g1_d[:, ck * cw:(ck + 1) * cw]
                )
                cw2 = (F // 2) * H // 4
                nc.sync.dma_start(
                    out=g2_sb[:, ck * cw2:(ck + 1) * cw2],
                    in_=g2_d[:, ck * cw2:(ck + 1) * cw2],
                )

        # ---------- MLP ----------
        with ExitStack() as mlp_ctx:
            zkeep = mlp_ctx.enter_context(tc.tile_pool(name="zkeep", bufs=1))
            zall = zkeep.tile([128, F, BC], bf16)
            qsb = zkeep.tile([F // 2, 2, BC], bf16)
            bsum = zkeep.tile([1, BC], bf16)

            # ----- pass 1: gather + fused fc1 (fp8 DoubleRow) + relu + stats
            m_ctx = mlp_ctx.enter_context(ExitStack())
            if True:
                mconst = m_ctx.enter_context(tc.tile_pool(name="mconst", bufs=1))
                gpool = m_ctx.enter_context(tc.tile_pool(name="gpool", bufs=2))
                mpsum = m_ctx.enter_context(tc.tile_pool(name="mpsum", bufs=3, space="PSUM"))

                FH = F // 2
                stm = m_ctx.enter_context(tc.tile_pool(name="stm", bufs=1, space="PSUM"))
                s2h0 = stm.tile([FH, BC], f32, tag="s2h0", name="s2h0")
                s2h1 = stm.tile([FH, BC], f32, tag="s2h1", name="s2h1")
                q2h0 = stm.tile([FH, BC], f32, tag="q2h0", name="q2h0")
                q2h1 = stm.tile([FH, BC], f32, tag="q2h1", name="q2h1")
                s2h = [s2h0, s2h1]
                q2h = [q2h0, q2h1]

                for call in range(NCALLS):
                    gout = gpool.tile([128, 2 * GF_CALL, BC], bf16, tag="gout")
                    nidx = GF_CALL * 256
                    nc.gpsimd.dma_gather(
                        out_ap=gout[:],
                        in_ap=ew_dram[:],
                        idxs_ap=gidx[:, call * GF_CALL * 16:(call + 1) * GF_CALL * 16],
                        num_idxs=nidx,
                        num_idxs_reg=nidx,
                        elem_size=BC,
                    )
                    for jf in range(GF_CALL):
                        f = call * GF_CALL + jf
                        z_ps = mpsum.tile([H, BC], f32, tag="zps")
                        nc.tensor.matmul(
                            out=z_ps[:], lhsT=g1_sb[:, f * H:(f + 1) * H],
                            rhs=gout[:, 2 * jf, :], start=True, stop=False,
                        )
                        nc.tensor.matmul(
                            out=z_ps[:],
                            lhsT=g2_sb[64 * (f % 2):64 * (f % 2) + 32,
                                       (f // 2) * H:(f // 2 + 1) * H],
                            rhs=gout[64 * (f % 2):64 * (f % 2) + 32, 2 * jf + 1, :],
                            start=False, stop=True,
                        )
                        z_f = zall[:, f, :]
                        if f % 2 == 0:
                            nc.scalar.activation(out=z_f, in_=z_ps[:], func=AF.Relu)
                        else:
                            nc.vector.tensor_scalar_max(out=z_f, in0=z_ps[:], scalar1=0.0)
                        zsq = gpool.tile([H, BC], bf16, tag="zsq")
                        nc.vector.tensor_tensor(out=zsq[:], in0=z_f, in1=z_f, op=OP.mult)
                        half = f // FH
                        nc.tensor.matmul(
                            out=s2h[half][:], lhsT=ohcol_sb[:, f * FH:(f + 1) * FH],
                            rhs=z_f, start=(f % FH == 0), stop=(f % FH == FH - 1),
                        )
                        nc.tensor.matmul(
                            out=q2h[half][:], lhsT=ohcol_sb[:, f * FH:(f + 1) * FH],
                            rhs=zsq[:], start=(f % FH == 0), stop=(f % FH == FH - 1),
                        )

            # ----- interlude: batched rstd2 from in-pass1 stats -----
            with ExitStack() as i_ctx:
                iwork = i_ctx.enter_context(tc.tile_pool(name="iwork", bufs=1))
                ipsum = i_ctx.enter_context(tc.tile_pool(name="ipsum", bufs=1, space="PSUM"))

                gam = iwork.tile([F // 2, 2, BC], bf16, tag="gam")
                FH = F // 2
                scp2is, t2s = [], []
                for i in range(2):
                    scp2i = iwork.tile([FH, BC], f32, tag=f"iscp2_{i}", name=f"iscp2_{i}")
                    nc.scalar.activation(out=scp2i[:], in_=s2h[i][:], func=AF.Copy)
                    u2 = iwork.tile([FH, BC], f32, tag=f"u2_{i}", name=f"u2_{i}")
                    nc.vector.scalar_tensor_tensor(
                        out=u2[:], in0=scp2i[:], scalar=-1.0 / H, in1=scp2i[:],
                        op0=OP.mult, op1=OP.mult,
                    )
                    t2 = iwork.tile([FH, BC], f32, tag=f"t2_{i}", name=f"t2_{i}")
                    nc.vector.tensor_tensor(out=t2[:], in0=u2[:], in1=q2h[i][:], op=OP.add)
                    nc.vector.tensor_scalar_max(out=t2[:], in0=t2[:], scalar1=0.0)
                    scp2is.append(scp2i)
                    t2s.append(t2)
                for i in range(2):
                    nc.scalar.activation(out=t2s[i][:], in_=t2s[i][:], func=AF.Ln, bias=epsH[:FH])
                for i in range(2):
                    nc.scalar.activation(out=qsb[:, i, :], in_=t2s[i][:], func=AF.Exp, scale=-0.5)
                    qf32 = iwork.tile([FH, BC], f32, tag=f"qf32_{i}", name=f"qf32_{i}")
                    nc.vector.tensor_copy(out=qf32[:], in_=qsb[:, i, :])
                    nc.vector.scalar_tensor_tensor(
                        out=gam[:, i, :], in0=scp2is[i][:], scalar=1.0 / H, in1=qf32[:],
                        op0=OP.mult, op1=OP.mult,
                    )
                ones50 = iwork.tile([F // 2, 1], bf16, tag="o50")
                nc.vector.memset(ones50[:], 1.0)
                bs_ps = ipsum.tile([1, BC], f32, tag="bsps")
                nc.tensor.matmul(
                    out=bs_ps[:], lhsT=ones50[:], rhs=gam[:, 0, :], start=True, stop=False
                )
                nc.tensor.matmul(
                    out=bs_ps[:], lhsT=ones50[:], rhs=gam[:, 1, :], start=False, stop=True
                )
                nc.scalar.activation(out=bsum[:], in_=bs_ps[:], func=AF.Copy)

            m_ctx.close()

            sqpool = mlp_ctx.enter_context(tc.tile_pool(name="sqpool", bufs=1))
            selq_sb = sqpool.tile([F // 2, F * 128], bf16)
            nc.sync.dma_start(out=selq_sb[:], in_=selq_d)

            # ----- pass 2: scale z (fp8), fc2 DoubleRow over pairs, output -----
            with ExitStack() as p2_ctx:
                p2const = p2_ctx.enter_context(tc.tile_pool(name="p2const", bufs=1))
                p2psum = p2_ctx.enter_context(tc.tile_pool(name="p2psum", bufs=3, space="PSUM"))
                ypsum = p2_ctx.enter_context(tc.tile_pool(name="ypsum", bufs=1, space="PSUM"))
                p2work = p2_ctx.enter_context(tc.tile_pool(name="p2work", bufs=3))

                y_ps0 = ypsum.tile([C, BC], f32, tag="y0", name="y0")
                y_ps1 = ypsum.tile([C, BC], f32, tag="y1", name="y1")
                y_two = [y_ps0, y_ps1]
                for f in range(F):
                    qb_ps = p2psum.tile([128, BC], f32, tag="qbps")
                    nc.tensor.matmul(
                        out=qb_ps[:], lhsT=selq_sb[:, f * 128:(f + 1) * 128],
                        rhs=qsb[:, f // (F // 2), :], start=True, stop=True,
                    )
                    zsc = p2work.tile([128, BC], bf16, tag="zsc")
                    if f % 3 != 2:
                        nc.vector.tensor_tensor(
                            out=zsc[:], in0=zall[:, f, :], in1=qb_ps[:], op=OP.mult
                        )
                    else:
                        qbc = p2work.tile([128, BC], bf16, tag="qbc")
                        nc.scalar.activation(out=qbc[:], in_=qb_ps[:], func=AF.Copy)
                        nc.gpsimd.tensor_tensor(
                            out=zsc[:], in0=zall[:, f, :], in1=qbc[:], op=OP.mult
                        )
                    nc.tensor.matmul(
                        out=y_two[f % 2][:], lhsT=w2pp[:], rhs=zsc[:],
                        start=(f < 2), stop=False,
                    )
                nc.tensor.matmul(
                    out=y_ps0[:], lhsT=negcs2[:], rhs=bsum[:], start=False, stop=True
                )
                nc.tensor.matmul(
                    out=y_ps1[:], lhsT=b2ppx[:], rhs=onesrow[:], start=False, stop=True
                )
                y0s = p2work.tile([C, BC], f32, tag="y0s")
                nc.scalar.activation(out=y0s[:], in_=y_ps0[:], func=AF.Copy, scale=1.0 / F)
                y1s = p2work.tile([C, BC], f32, tag="y1s")
                nc.scalar.activation(out=y1s[:], in_=y_ps1[:], func=AF.Copy, scale=1.0 / F)
                ysb = p2work.tile([C, BC], f32, tag="ysb")
                nc.vector.tensor_tensor(
                    out=ysb[:], in0=y0s[:], in1=y1s[:], op=OP.add
                )
                for bt in range(BC // 128):
                    yt_ps = p2psum.tile([128, C], f32, tag="ytps")
                    nc.tensor.transpose(
                        out=yt_ps[:], in_=ysb[:, bt * 128:(bt + 1) * 128],
                        identity=ident[0:C, 0:C],
                    )
                    yt = p2work.tile([128, C], f32, tag="yt")
                    nc.vector.tensor_copy(out=yt[:], in_=yt_ps[:])
                    nc.sync.dma_start(out=y_d[bt * 128:(bt + 1) * 128, :], in_=yt[:])

    nc.compile()
    return nc


_CACHED = {}


def _get_program():
    if "nc" not in _CACHED:
        _CACHED["nc"] = _build_program()
    return _CACHED["nc"]


_LAST_RESULTS = None


def kernel(**inputs):
    global _LAST_RESULTS
    dev, x = _host_prep(inputs)
    nc = _get_program()

    in_maps = []
    for cid in range(NCORES):
        m = dict(dev)
        m["x_shard"] = np.ascontiguousarray(x[cid * BC:(cid + 1) * BC])
        in_maps.append(m)

    res = bass_utils.run_bass_kernel_spmd(nc, in_maps, core_ids=list(range(NCORES)))
    _LAST_RESULTS = res
    y = np.concatenate([r["y_out"] for r in res.results], axis=0)
    return y.astype(np.float32)


if __name__ == "__main__":
    # CoreSim smoke test on one core
    sys.path.insert(0, "/root/problem")
    import jax
    import reference

    with jax.default_device(jax.devices("cpu")[0]):
        inputs = {k: np.asarray(v) for k, v in reference.setup_inputs().items()}
    dev, x = _host_prep(inputs)
    nc = _build_program()
    from concourse.bass_interp import CoreSim

    sim = CoreSim(nc, trace=False)
    for k, v in dev.items():
        sim.tensor(k)[:] = v
    sim.tensor("x_shard")[:] = x[:BC]
    sim.simulate(check_with_hw=False)
    y0 = np.array(sim.tensor("y_out"))
    with jax.default_device(jax.devices("cpu")[0]):
        exp = np.asarray(reference.reference(**inputs))[:BC]
    err = np.abs(y0 - exp).max()
    rel2 = np.linalg.norm(y0 - exp) / (np.linalg.norm(exp) + 1e-30)
    print("sim maxabs:", err, " rel-l2:", rel2)



# revision 29
# speedup vs baseline: 1.5459x; 1.5459x over previous
"""DOFEN forward kernel for 8x Trainium2 NeuronCores (pure batch data-parallel).

Contract: kernel(**inputs) takes the FULL inputs from setup_inputs() and
returns the FULL [4096, 10] float32 output.

v3 design (per core, feature-partition layout [feat, batch], BC=512):
  P1:  O = tanh(0.5*U + bhat) via PE matmul + ACT; group sums s1 (PE,
       tile_position quadrants) and raw square-sums q1 = sum(O^2); scp1
       (group means) now on DVE so the ACT engine runs ONLY tanh in P1 ->
       exactly two act-table loads in the whole program.
  P1b: one fused Ln + Exp over all packs -> r1.
  P3:  conv2 with folded GN1 mean correction; t = c2 * broadcast(r1);
       h = relu(t + b2); GN2 stats; conv3 with mean correction; per-pack
       ew = exp(w + b3 - 4) stored to DRAM pack-by-pack so the gather
       can start right after the last pack (no monolithic store stall).
  MLP (restructured around the "output-free-dim is what matmuls cost"
       property of TRN2):
       - gather: 12 forests/call (15x128 descriptors; ring enlarged to
         4096 descs), zero padding rows: tails of 4 forests share a tile.
       - fc1: two chained matmuls per forest (or one fp8 DoubleRow when
         KFP8G=1) -> z [128, BC] per forest; relu split ACT/DVE in
         2-forest pairs; zsq on DVE (4x mode).
       - LN2 stats per (forest, batch-tile) via TINY transposed matmuls:
         lhsT = z slice [128H, 128b], rhs = ones column -> out [128b, 1]
         at ~1ns each. Stats live as columns of per-bt PSUM tiles.
       - interlude per group of 20 forests: rstd2 = 1/sqrt(var+eps) in
         [128b, 20] layout (small DVE/ACT ops), fused gam+gamsum via
         tensor_tensor_reduce.
       - fc2 transposed: per (f, bt) one matmul lhsT = z slice, rhs =
         W2p [128, 10] -> out [128b, 10] in PSUM at ~5ns; scale by rstd
         with a broadcast tensor_tensor; accumulate + tree-sum; final
         mean correction is a rank-1 update. Output is already
         batch-major: direct DMA, no transposes.

Hardware-legality notes: DVE tensor ops may read at most ONE PSUM
operand, Pool/GPSIMD must stay SBUF-only, a single dma_gather must not
exceed the SWDGE ring (4096 descs here).
"""

import os
import sys

for _p in ("/opt/trn_rl_repo", "/root/.axon_site/_ro/trn_rl_repo"):
    if os.path.isdir(_p) and _p not in sys.path:
        sys.path.insert(0, _p)

import numpy as np
import ml_dtypes

import concourse.bass as bass
import concourse.bacc as bacc
import concourse.tile as tile
from concourse import mybir
import concourse.bass_utils as bass_utils

# ---- problem shapes (hardcoded per contest contract) ----
B = 4096
NCOL = 100
NCOND = 64
D = 4
TOTAL = 6400           # n_col * n_cond
G = 1600               # n_rodt groups
NEST = 160
F = 100                # forests
H = 128                # hidden
C = 10                 # classes
EPS = 1e-5
NCORES = 8
BC = B // NCORES       # 512 per core
NT = TOTAL // 128      # 50 feature tiles
NPACK = (NT + 3) // 4  # 13 packed stats tiles (last covers 2 src tiles)
GPAD = NPACK * 128     # 1664 padded rodt rows
LN2C = float(np.log(2.0))

GF = 6                             # forests per dma_gather call (6 mains +
                                   # 2 tail tiles = 1024 descs = HW ring cap)
CALL_F0 = list(range(0, F, GF))    # call start forests
CALLS = [(f0, min(GF, F - f0)) for f0 in CALL_F0]


def _call_slot(f):
    """Call-relative tail slot for forest f (row offset 32*slot in its
    shared tail tile)."""
    f0 = (f // GF) * GF
    return (f - f0) % 4
GRP = 20                           # forests per LN2-stat group
NGRP = F // GRP

KFP8G = os.environ.get("KFP8G", "0") == "1"   # fp8 fc1 weights (measured 4.4e-2 rel err: too lossy)

f32 = mybir.dt.float32
bf16 = mybir.dt.float16   # 16-bit activations/weights use fp16 (11-bit mantissa)
fp8 = mybir.dt.float8e4
i16 = mybir.dt.int16
AF = mybir.ActivationFunctionType
OP = mybir.AluOpType
PM = mybir.MatmulPerfMode

BF = np.float16
F8 = ml_dtypes.float8_e4m3


def _host_prep(inputs):
    """Fold all parameter algebra on the host; returns dict of device arrays."""
    f64 = np.float64
    x = np.asarray(inputs["x"], np.float32)
    w1 = np.asarray(inputs["w1"], f64)
    b1 = np.asarray(inputs["b1"], f64)
    perm = np.asarray(inputs["perm"], np.int64)
    gn1_w = np.asarray(inputs["gn1_w"], f64)
    gn1_b = np.asarray(inputs["gn1_b"], f64)
    conv2_w = np.asarray(inputs["conv2_w"], f64)
    conv2_b = np.asarray(inputs["conv2_b"], f64)
    gn2_w = np.asarray(inputs["gn2_w"], f64)
    gn2_b = np.asarray(inputs["gn2_b"], f64)
    conv3_w = np.asarray(inputs["conv3_w"], f64)
    conv3_b = np.asarray(inputs["conv3_b"], f64)
    swr = np.asarray(inputs["swr"], np.int64)
    E = np.asarray(inputs["E"], f64)
    ln1_w = np.asarray(inputs["ln1_w"], f64)
    ln1_b = np.asarray(inputs["ln1_b"], f64)
    fc1_w = np.asarray(inputs["fc1_w"], f64)
    fc1_b = np.asarray(inputs["fc1_b"], f64)
    ln2_w = np.asarray(inputs["ln2_w"], f64)
    ln2_b = np.asarray(inputs["ln2_b"], f64)
    fc2_w = np.asarray(inputs["fc2_w"], f64)
    fc2_b = np.asarray(inputs["fc2_b"], f64)

    dev = {}

    # --- O-matmul weights: feature t corresponds to reference feature perm[t]
    p = perm
    c_arr = p // NCOL
    j_arr = p % NCOL
    What = np.zeros((100, TOTAL), np.float32)
    What[j_arr, np.arange(TOTAL)] = w1[j_arr, c_arr]
    dev["what"] = What.astype(BF)
    # folded into the tanh bias: tanh(0.5*U + 0.5*bhat)
    dev["bhat"] = (0.5 * b1[j_arr, c_arr]).reshape(NT, 128).T.astype(np.float32).copy()

    # --- GN1 affine folded into conv2 weights/bias ---
    w2f = conv2_w * gn1_w.reshape(G, D, 1)                 # [G, D, D]
    b2f = conv2_b.reshape(G, D) + np.einsum("gi,gio->go", gn1_b.reshape(G, D), conv2_w)
    bd2 = np.zeros((128, NT * 128), np.float32)
    gi = np.arange(32)
    for t in range(NT):
        g0 = 32 * t
        for i in range(D):
            for o in range(D):
                bd2[4 * gi + i, t * 128 + 4 * gi + o] = w2f[g0 + gi, i, o]
    dev["bd2"] = bd2.astype(BF)

    # conv2 group-mean correction: chained matmul  c2 -= m2corr^T scp1,
    # scp1[32j+u] = mean of group 32t+u (j = t%4).
    m2corr = np.zeros((128, NT * 128), np.float32)
    w2s = w2f.sum(1)                                       # [G, D] sum over i
    for t in range(NT):
        j = t % 4
        pp = np.arange(128)
        m2corr[32 * j + pp // 4, t * 128 + pp] = -w2s[32 * t + pp // 4, pp % 4]
    dev["m2corr"] = m2corr.astype(BF)

    # --- GN2 + conv3 ---
    w3f = conv3_w[:, :, 0] * gn2_w.reshape(G, D)
    b3f = conv3_b + (gn2_b.reshape(G, D) * conv3_w[:, :, 0]).sum(1)
    bd3 = np.zeros((128, NT * 32), np.float32)
    for t in range(NT):
        g0 = 32 * t
        for i in range(D):
            bd3[4 * gi + i, t * 32 + gi] = w3f[g0 + gi, i]
    dev["bd3"] = bd3.astype(BF)

    # conv3 mean correction scalars: u = scp2 * sw3col, then c3 -= I u
    sw3 = w3f.sum(1)                                       # [G]
    sw3_pad = np.zeros(GPAD, np.float64)
    sw3_pad[:G] = sw3
    dev["sw3col"] = sw3_pad.reshape(NPACK, 128).T.astype(np.float32).copy()

    # ones-block for group sums: [128, 32]
    onesblk = np.zeros((128, 32), np.float32)
    onesblk[np.arange(128), np.arange(128) // 4] = 1.0
    dev["onesblk"] = onesblk.astype(BF)

    # group-stat broadcast selectors: bcastj[k, j*128+m] = (k == 32*j + m//4)
    bcastj = np.zeros((128, 4 * 128), np.float32)
    for j in range(4):
        m = np.arange(128)
        bcastj[32 * j + m // 4, j * 128 + m] = 1.0
    dev["bcastj"] = bcastj.astype(BF)

    # bias columns
    b2f_col = np.zeros((128, NT), np.float32)
    pp = np.arange(128)
    for t in range(NT):
        b2f_col[pp, t] = b2f[32 * t + pp // 4, pp % 4]
    dev["b2fcol"] = b2f_col
    b3f_pad = np.zeros(GPAD, np.float64)
    b3f_pad[:G] = b3f
    dev["b3fcol"] = (b3f_pad - 4.0).reshape(NPACK, 128).T.astype(np.float32).copy()

    # --- MLP folds ---
    W1p = ln1_w[:, None] * fc1_w                           # [128, 128]
    b1p = fc1_b + ln1_b @ fc1_w
    if np.abs(b1p).max() > 1e-12:
        raise NotImplementedError(
            "fused kernel assumes fc1_b + ln1_b@fc1_w == 0 (true for this problem)"
        )
    Ep = E[swr]                                            # [F, 160, 128]
    muEp = Ep.mean(2)                                      # [F, 160]
    cs1 = W1p.sum(0)                                       # [128]
    Gall = np.einsum("fed,dh->feh", Ep, W1p) - muEp[:, :, None] * cs1[None, None, :]

    # gpack: [128, F, 2, H]; k-tile 0 = est rows 0..127, k-tile 1 = est rows
    # 128..159 placed at partitions 32*(f%4).. (matching the shared tail tile)
    gpack = np.zeros((128, F, 2, H), np.float64)
    for f in range(F):
        a = _call_slot(f)
        gpack[:, f, 0, :] = Gall[f, :128, :]
        gpack[32 * a:32 * a + 32, f, 1, :] = Gall[f, 128:160, :]
    dev["gpack"] = gpack.astype(F8 if KFP8G else BF)

    W2p = ln2_w[:, None] * fc2_w                           # [128, 10]
    b2p = fc2_b + ln2_b @ fc2_w                            # [10]
    if np.abs(b2p).max() > 1e-12:
        raise NotImplementedError("fused kernel assumes fc2_b + ln2_b@fc2_w == 0")
    cs2 = W2p.sum(0)                                       # [10]
    dev["w2p"] = W2p.astype(BF)
    dev["cs2f"] = np.tile((cs2 / F).astype(np.float32)[None, :], (128, 1))

    # gather indices: per call of nf forests -> nf main tiles (128 est rows)
    # then ceil(nf/4) tail tiles (4 forests x 32 tail rows each)
    idx_cols = []
    for f0, nf in CALLS:
        L = []
        for f in range(f0, f0 + nf):
            L.extend(swr[f, :128].tolist())
        for tb in range((nf + 3) // 4):
            for f in range(f0 + 4 * tb, f0 + 4 * tb + 4):
                if f < f0 + nf:
                    L.extend(swr[f, 128:160].tolist())
                else:
                    L.extend([0] * 32)
        L = np.asarray(L, np.int16)
        wrap = L.reshape(-1, 16).T
        idx_cols.append(np.tile(wrap, (8, 1)))
    dev["gidx"] = np.concatenate(idx_cols, axis=1)  # [128, sum(desc)/16] int16

    return dev, x


def _patched_act_tables(orig_fn):
    """Strip the shared funcs from all but one superset table so the
    act-table-load pass keeps a single table for everything after tanh."""
    def wrapper(arch):
        tabs = {k: set(v) for k, v in orig_fn(arch).items()}
        need = {AF.Ln, AF.Exp, AF.Relu, AF.Copy, AF.Square}
        s6 = None
        for name, s in tabs.items():
            if need <= s and AF.Tanh not in s:
                s6 = name
                break
        if s6 is None:
            return tabs
        for name in tabs:
            if name != s6:
                tabs[name] = tabs[name] - need
        return tabs
    return wrapper


def _build_program():
    """Build the per-core Bass program."""
    if os.environ.get("KNOACTPATCH", "0") == "1":
        return _build_program_inner()
    _orig_gat = bacc.get_activation_tables
    bacc.get_activation_tables = _patched_act_tables(_orig_gat)
    try:
        return _build_program_inner()
    finally:
        bacc.get_activation_tables = _orig_gat


def _build_program_inner():
    nc = bacc.Bacc("TRN2", debug=False, num_devices=NCORES)

    def din(name, shape, dt):
        return nc.dram_tensor(name, list(shape), dt, kind="ExternalInput").ap()

    NIDXCOL = sum((nf + (nf + 3) // 4) * 8 for _, nf in CALLS)
    x_d = din("x_shard", [BC, NCOL], f32)
    what_d = din("what", [100, TOTAL], bf16)
    bhat_d = din("bhat", [128, NT], f32)
    bd2_d = din("bd2", [128, NT * 128], bf16)
    m2corr_d = din("m2corr", [128, NT * 128], bf16)
    bd3_d = din("bd3", [128, NT * 32], bf16)
    sw3col_d = din("sw3col", [128, NPACK], f32)
    onesblk_d = din("onesblk", [128, 32], bf16)
    bcastj_d = din("bcastj", [128, 4 * 128], bf16)
    b2fcol_d = din("b2fcol", [128, NT], f32)
    b3fcol_d = din("b3fcol", [128, NPACK], f32)
    gpack_d = din("gpack", [128, F, 2, H], fp8 if KFP8G else bf16)
    w2p_d = din("w2p", [H, C], bf16)
    cs2f_d = din("cs2f", [128, C], f32)
    gidx_d = din("gidx", [128, NIDXCOL], i16)

    y_d = nc.dram_tensor("y_out", [BC, C], f32, kind="ExternalOutput").ap()

    from contextlib import ExitStack
    from concourse.masks import make_identity

    with tile.TileContext(nc) as tc, ExitStack() as ctx:
        persist = ctx.enter_context(tc.tile_pool(name="persist", bufs=1))
        dram_pool = ctx.enter_context(tc.tile_pool(name="drams", bufs=1, space="DRAM"))

        ident = persist.tile([128, 128], f32)
        make_identity(nc, ident[:])
        eps4 = persist.tile([128, 1], f32)
        nc.vector.memset(eps4[:], 4.0 * EPS)
        ln2b = persist.tile([128, 1], f32)
        nc.vector.memset(ln2b[:], LN2C)
        onecol = persist.tile([128, 1], bf16)
        nc.vector.memset(onecol[:], 1.0)
        epsc = persist.tile([128, 1], f32)
        nc.vector.memset(epsc[:], EPS)

        # MLP consts: tiles allocated up front, DMAs issued later so the
        # phi2-critical loads (x, what) go first in the queue.  gpack is
        # split: first half lives through phi2 (loaded during P1b when DMA
        # is idle), second half loads at MLP start into freed phi2 space.
        gidx = persist.tile([128, NIDXCOL], i16)
        w2p_sb = persist.tile([H, C], bf16)
        cs2f_sb = persist.tile([128, C], f32)
        gdt = fp8 if KFP8G else bf16
        FA = F // 2
        gpackA = persist.tile([128, FA, 2, H], gdt)

        ew_dram = dram_pool.tile([GPAD, BC], bf16)

        # ---------- phi2 ----------
        with ExitStack() as phi_ctx:
            pconst = phi_ctx.enter_context(tc.tile_pool(name="pconst", bufs=1))
            onesblk = pconst.tile([128, 32], bf16)
            bcastj = pconst.tile([128, 4 * 128], bf16)
            b2fcol = pconst.tile([128, NT], f32)
            b3fcol = pconst.tile([128, NPACK], f32)
            sw3col = pconst.tile([128, NPACK], f32)
            bd2_sb = pconst.tile([128, NT * 128], bf16)

            ofall = pconst.tile([128, NT, BC], bf16)
            scp1_all = pconst.tile([128, NPACK, BC], bf16)
            r1_all = pconst.tile([128, NPACK, BC], bf16)
            # conv2-correction + conv3 weights: tiles allocated now (pool
            # stack order), DMAs issued after P1's critical loads
            pBconst = phi_ctx.enter_context(tc.tile_pool(name="pBconst", bufs=1))
            m2corr_sb = pBconst.tile([128, NT * 128], bf16)
            bd3_sb = pBconst.tile([128, NT * 32], bf16)
            qpool_ctx = ExitStack()
            qpool = qpool_ctx.enter_context(tc.tile_pool(name="qpool", bufs=1))
            qall = qpool.tile([128, NPACK, BC], bf16)
            nc.vector.memset(qall[64:128, NPACK - 1, :], 1.0)

            # ---- P1 (ACT set: tanh only): O, group sums, square sums
            with ExitStack() as sA:
                pA = sA.enter_context(tc.tile_pool(name="pA", bufs=1))
                xts = []
                for bt in range(BC // 128):
                    x_t = pA.tile([128, NCOL], f32, tag="xload", name=f"xl{bt}")
                    nc.sync.dma_start(out=x_t[:], in_=x_d[bt * 128:(bt + 1) * 128, :])
                    xts.append(x_t)
                what_sb = pA.tile([100, TOTAL], bf16)
                # chunked so tile 0's matmul can start early
                WCH = 4
                for ck in range(WCH):
                    cw = TOTAL // WCH
                    nc.sync.dma_start(
                        out=what_sb[:, ck * cw:(ck + 1) * cw],
                        in_=what_d[:, ck * cw:(ck + 1) * cw],
                    )
                bhat_sb = pA.tile([128, NT], f32)
                nc.sync.dma_start(out=bhat_sb[:], in_=bhat_d)
                nc.sync.dma_start(out=onesblk[:], in_=onesblk_d)
                nc.sync.dma_start(out=bcastj[:], in_=bcastj_d)
                nc.sync.dma_start(out=b2fcol[:], in_=b2fcol_d)
                nc.sync.dma_start(out=b3fcol[:], in_=b3fcol_d)
                nc.sync.dma_start(out=sw3col[:], in_=sw3col_d)
                # deferred heavy/late consts, behind the phi2-critical loads
                nc.sync.dma_start(out=bd2_sb[:], in_=bd2_d)
                nc.sync.dma_start(out=gidx[:], in_=gidx_d)
                nc.sync.dma_start(out=w2p_sb[:], in_=w2p_d)
                nc.sync.dma_start(out=cs2f_sb[:], in_=cs2f_d)

                x_aug = pA.tile([100, BC], bf16)
                with tc.tile_pool(name="xtp", bufs=2, space="PSUM") as xtpool:
                    for bt in range(BC // 128):
                        x_ps = xtpool.tile([NCOL, 128], f32, tag="xtps")
                        nc.tensor.transpose(out=x_ps[:], in_=xts[bt][:], identity=ident[:])
                        nc.vector.tensor_copy(
                            out=x_aug[0:NCOL, bt * 128:(bt + 1) * 128], in_=x_ps[:]
                        )

                popool = sA.enter_context(tc.tile_pool(name="popool", bufs=2, space="PSUM"))
                st1 = sA.enter_context(tc.tile_pool(name="st1", bufs=2, space="PSUM"))
                osqp = sA.enter_context(tc.tile_pool(name="osqp", bufs=3))
                sq1p = sA.enter_context(tc.tile_pool(name="sq1p", bufs=2))

                for st in range(NPACK):
                    tiles = list(range(4 * st, min(4 * st + 4, NT)))
                    nrow = 32 * len(tiles)
                    s1P = st1.tile([128, BC], f32, tag="s1")
                    q1P = st1.tile([128, BC], f32, tag="q1")
                    for t in tiles:
                        j = t % 4
                        po = popool.tile([128, BC], f32, tag="po")
                        nc.tensor.matmul(
                            out=po[:], lhsT=what_sb[:, t * 128:(t + 1) * 128],
                            rhs=x_aug[:], start=True, stop=True,
                        )
                        o_t = ofall[:, t, :]
                        nc.scalar.activation(
                            out=o_t, in_=po[:], func=AF.Tanh, scale=0.5,
                            bias=bhat_sb[:, t:t + 1],
                        )
                        nc.tensor.matmul(
                            out=s1P[32 * j:32 * j + 32, :], lhsT=onesblk[:],
                            rhs=o_t, start=True, stop=True,
                            tile_position=(0, 32 * j),
                        )
                        osq = osqp.tile([128, BC], bf16, tag="osq")
                        if t % 5 < 2:
                            nc.gpsimd.tensor_tensor(out=osq[:], in0=o_t, in1=o_t, op=OP.mult)
                        else:
                            nc.vector.tensor_tensor(out=osq[:], in0=o_t, in1=o_t, op=OP.mult)
                        nc.tensor.matmul(
                            out=q1P[32 * j:32 * j + 32, :], lhsT=onesblk[:],
                            rhs=osq[:], start=True, stop=True,
                            tile_position=(0, 32 * j),
                        )
                    # scp1 = s/4 (group mean) on DVE; q -= 4*scp1^2
                    nc.vector.tensor_scalar_mul(
                        out=scp1_all[:nrow, st, :], in0=s1P[:nrow], scalar1=0.25
                    )
                    s1sq = sq1p.tile([128, BC], bf16, tag="s1sq")
                    nc.vector.tensor_tensor(
                        out=s1sq[:nrow], in0=scp1_all[:nrow, st, :],
                        in1=scp1_all[:nrow, st, :], op=OP.mult,
                    )
                    s1sqm = sq1p.tile([128, BC], bf16, tag="s1sqm")
                    nc.vector.tensor_scalar_mul(
                        out=s1sqm[:nrow], in0=s1sq[:nrow], scalar1=-4.0
                    )
                    qfix = sq1p.tile([128, BC], bf16, tag="qfix")
                    nc.vector.tensor_tensor(
                        out=qfix[:nrow], in0=s1sqm[:nrow], in1=q1P[:nrow], op=OP.add
                    )
                    # clamp >= 0 (f16 rounding can push sum(x^2) - s^2/4
                    # slightly negative -> Ln NaN)
                    nc.vector.tensor_scalar_max(
                        out=qall[:nrow, st, :], in0=qfix[:nrow], scalar1=0.0
                    )

            # conv2-correction + conv3 weights + MLP weights: loaded while
            # P1b's ln/exp run (DMA is idle here)
            nc.sync.dma_start(out=m2corr_sb[:], in_=m2corr_d)
            nc.sync.dma_start(out=bd3_sb[:], in_=bd3_d)
            for ck in range(2):
                cf = FA // 2
                nc.sync.dma_start(
                    out=gpackA[:, ck * cf:(ck + 1) * cf, :, :],
                    in_=gpack_d[:, ck * cf:(ck + 1) * cf, :, :],
                )

            # ---- P1b: GN1 rstd -- one fused Ln, one fused Exp
            nc.scalar.activation(out=qall[:], in_=qall[:], func=AF.Ln, bias=eps4[:])
            nc.scalar.activation(
                out=r1_all[:], in_=qall[:], func=AF.Exp, scale=-0.5, bias=ln2b[:],
            )
            qpool_ctx.close()

            # ---- P3: conv2 + relu + GN2 stats + conv3 + exp, per pack; the
            # ew pack rows go to DRAM immediately so gathers can start early.
            with ExitStack() as sB:
                c2pool = sB.enter_context(tc.tile_pool(name="c2pool", bufs=2, space="PSUM"))
                rbpool = sB.enter_context(tc.tile_pool(name="rbpool", bufs=2, space="PSUM"))
                st2 = sB.enter_context(tc.tile_pool(name="st2", bufs=1, space="PSUM"))
                c3pool = sB.enter_context(tc.tile_pool(name="c3pool", bufs=2, space="PSUM"))
                tpool = sB.enter_context(tc.tile_pool(name="tpool", bufs=2))
                hpool = sB.enter_context(tc.tile_pool(name="hpool", bufs=1))
                hsqp = sB.enter_context(tc.tile_pool(name="hsqp", bufs=2))
                packp = sB.enter_context(tc.tile_pool(name="packp", bufs=1))
                ewkeep = sB.enter_context(tc.tile_pool(name="ewkeep", bufs=1))
                ew_sb = ewkeep.tile([128, NPACK, BC], bf16)

                for st in range(NPACK):
                    tiles = list(range(4 * st, min(4 * st + 4, NT)))
                    nrow = 32 * len(tiles)
                    s2P = st2.tile([128, BC], f32, tag="s2")
                    q2P = st2.tile([128, BC], f32, tag="q2")
                    c3P = c3pool.tile([128, BC], f32, tag="c3")
                    for t in tiles:
                        j = t % 4
                        c2p = c2pool.tile([128, BC], f32, tag="c2")
                        nc.tensor.matmul(
                            out=c2p[:], lhsT=bd2_sb[:, t * 128:(t + 1) * 128],
                            rhs=ofall[:, t, :], start=True, stop=False,
                        )
                        nc.tensor.matmul(
                            out=c2p[:], lhsT=m2corr_sb[0:nrow, t * 128:(t + 1) * 128],
                            rhs=scp1_all[0:nrow, st, :], start=False, stop=True,
                        )
                        rbp = rbpool.tile([128, BC], f32, tag="rb")
                        nc.tensor.matmul(
                            out=rbp[:], lhsT=bcastj[0:nrow, j * 128:(j + 1) * 128],
                            rhs=r1_all[0:nrow, st, :], start=True, stop=True,
                        )
                        c2s = tpool.tile([128, BC], bf16, tag="c2s")
                        nc.scalar.activation(out=c2s[:], in_=c2p[:], func=AF.Copy)
                        tsb = tpool.tile([128, BC], bf16, tag="tsb")
                        nc.vector.tensor_tensor(
                            out=tsb[:], in0=c2s[:], in1=rbp[:], op=OP.mult
                        )
                        # h = relu(t + b2) on DVE
                        hf = hpool.tile([128, BC], bf16, tag=f"h{j}")
                        nc.vector.tensor_scalar(
                            out=hf[:], in0=tsb[:], scalar1=b2fcol[:, t:t + 1],
                            scalar2=0.0, op0=OP.add, op1=OP.max,
                        )
                        nc.tensor.matmul(
                            out=s2P[32 * j:32 * j + 32, :], lhsT=onesblk[:],
                            rhs=hf[:], start=True, stop=True,
                            tile_position=(0, 32 * j),
                        )
                        hsq = hsqp.tile([128, BC], bf16, tag="hsq")
                        nc.gpsimd.tensor_tensor(out=hsq[:], in0=hf[:], in1=hf[:], op=OP.mult)
                        nc.tensor.matmul(
                            out=q2P[32 * j:32 * j + 32, :], lhsT=onesblk[:],
                            rhs=hsq[:], start=True, stop=True,
                            tile_position=(0, 32 * j),
                        )
                        nc.tensor.matmul(
                            out=c3P[32 * j:32 * j + 32, :],
                            lhsT=bd3_sb[:, t * 32:(t + 1) * 32],
                            rhs=hf[:], start=True, stop=True,
                            tile_position=(0, 32 * j),
                        )
                    # pack tail: GN2 rstd + conv3 corrections + exp, per pack
                    scp2 = packp.tile([128, BC], bf16, tag="scp2")
                    nc.scalar.activation(
                        out=scp2[:nrow], in_=s2P[:nrow], func=AF.Copy, scale=0.25,
                    )
                    s2sq = packp.tile([128, BC], bf16, tag="s2sq")
                    nc.vector.tensor_tensor(
                        out=s2sq[:nrow], in0=scp2[:nrow], in1=scp2[:nrow], op=OP.mult
                    )
                    s2sqm = packp.tile([128, BC], bf16, tag="s2sqm")
                    nc.vector.tensor_scalar_mul(
                        out=s2sqm[:nrow], in0=s2sq[:nrow], scalar1=-4.0
                    )
                    q2fix = packp.tile([128, BC], bf16, tag="q2fix")
                    nc.vector.tensor_tensor(
                        out=q2fix[:nrow], in0=s2sqm[:nrow], in1=q2P[:nrow], op=OP.add
                    )
                    nc.vector.tensor_scalar_max(
                        out=q2fix[:nrow], in0=q2fix[:nrow], scalar1=0.0
                    )
                    qln2 = packp.tile([128, BC], f32, tag="qln2")
                    nc.scalar.activation(
                        out=qln2[:nrow], in_=q2fix[:nrow], func=AF.Ln, bias=eps4[:nrow],
                    )
                    r2p = packp.tile([128, BC], bf16, tag="r2p")
                    nc.scalar.activation(
                        out=r2p[:nrow], in_=qln2[:nrow], func=AF.Exp, scale=-0.5,
                        bias=ln2b[:nrow],
                    )
                    u = packp.tile([128, BC], bf16, tag="u")
                    nc.vector.tensor_scalar(
                        out=u[:nrow], in0=scp2[:nrow], scalar1=sw3col[0:nrow, st:st + 1],
                        scalar2=None, op0=OP.mult,
                    )
                    v3 = packp.tile([128, BC], bf16, tag="v3")
                    nc.vector.tensor_tensor(
                        out=v3[:nrow], in0=c3P[:nrow], in1=u[:nrow], op=OP.subtract
                    )
                    wsb = packp.tile([128, BC], bf16, tag="wsb")
                    nc.vector.tensor_tensor(
                        out=wsb[:nrow], in0=v3[:nrow], in1=r2p[:nrow], op=OP.mult
                    )
                    nc.scalar.activation(
                        out=ew_sb[:nrow, st, :], in_=wsb[:nrow], func=AF.Exp,
                        bias=b3fcol[:nrow, st:st + 1],
                    )
                    if nrow < 128:
                        nc.vector.memset(ew_sb[nrow:128, st, :], 0.0)
                    # pack rows to DRAM immediately (row g = 128*st + p)
                    nc.sync.dma_start(
                        out=ew_dram[128 * st:128 * (st + 1), :], in_=ew_sb[:, st, :]
                    )

        # ---------- MLP ----------
        with ExitStack() as mlp_ctx:
            NBT = BC // 128
            zgpool = mlp_ctx.enter_context(tc.tile_pool(name="zgpool", bufs=3))
            gpool = mlp_ctx.enter_context(tc.tile_pool(name="gpool", bufs=2))
            gbpool = mlp_ctx.enter_context(tc.tile_pool(name="gbpool", bufs=1))
            gpackB = gbpool.tile([128, F - FA, 2, H], gdt)
            for ck in range(2):
                cf = (F - FA) // 2
                nc.sync.dma_start(
                    out=gpackB[:, ck * cf:(ck + 1) * cf, :, :],
                    in_=gpack_d[:, FA + ck * cf:FA + (ck + 1) * cf, :, :],
                )

            def gpack_at(f, kt):
                if f < FA:
                    return gpackA[:, f, kt, :]
                return gpackB[:, f - FA, kt, :]
            zps = mlp_ctx.enter_context(tc.tile_pool(name="zps", bufs=2, space="PSUM"))
            stq_pool = mlp_ctx.enter_context(tc.tile_pool(name="stq", bufs=1, space="PSUM"))
            zsqp = mlp_ctx.enter_context(tc.tile_pool(name="zsqp", bufs=3))
            iw = mlp_ctx.enter_context(tc.tile_pool(name="iw", bufs=2))
            ykeep = mlp_ctx.enter_context(tc.tile_pool(name="ykeep", bufs=1))

            # per-bt PSUM tiles: cols 0..F-1 = s stats, F..2F-1 = q stats,
            # 2F..2F+GRP*C-1 = pass-2 Y region (reused per group)
            stq = [stq_pool.tile([128, 2 * F + GRP * C], f32, name=f"stq{bt}")
                   for bt in range(NBT)]
            qT = [ykeep.tile([128, F], bf16, name=f"qT{bt}") for bt in range(NBT)]
            gparts = [ykeep.tile([128, NGRP], f32, name=f"gp{bt}") for bt in range(NBT)]
            yacc = [ykeep.tile([128, GRP * C], bf16, name=f"ya{bt}") for bt in range(NBT)]
            for bt in range(NBT):
                nc.vector.memset(yacc[bt][:], 0.0)

            zgrp = {}

            def get_zgrp(g):
                if g not in zgrp:
                    zgrp[g] = zgpool.tile([128, GRP, BC], bf16, tag="zg", name=f"zg{g}")
                return zgrp[g]

            # gather-call index column offsets
            call_cols = []
            cum = 0
            for f0, nf in CALLS:
                ntl = nf + (nf + 3) // 4
                call_cols.append((cum, ntl))
                cum += ntl * 8
            assert cum == NIDXCOL

            gouts = [None] * len(CALLS)

            def emit_gather(c):
                f0, nf = CALLS[c]
                col0, ntl = call_cols[c]
                gt = gpool.tile([128, GF + 2, BC], bf16, tag="gout", name=f"go{c}")
                nidx = ntl * 128
                nc.gpsimd.dma_gather(
                    out_ap=gt[:, 0:ntl, :],
                    in_ap=ew_dram[:],
                    idxs_ap=gidx[:, col0:col0 + ntl * 8],
                    num_idxs=nidx,
                    num_idxs_reg=nidx,
                    elem_size=BC,
                )
                gouts[c] = gt

            def emit_interlude(g):
                # rstd2 for forests [g*GRP, (g+1)*GRP) in [128b, GRP] layout
                for bt in range(NBT):
                    sc = stq[bt][:, g * GRP:(g + 1) * GRP]
                    qc = stq[bt][:, F + g * GRP:F + (g + 1) * GRP]
                    muT = iw.tile([128, GRP], bf16, tag="muT")
                    nc.vector.tensor_scalar_mul(out=muT[:], in0=sc, scalar1=1.0 / H)
                    musq = iw.tile([128, GRP], bf16, tag="musq")
                    nc.vector.tensor_tensor(out=musq[:], in0=muT[:], in1=muT[:], op=OP.mult)
                    vT = iw.tile([128, GRP], f32, tag="vT")
                    nc.vector.scalar_tensor_tensor(
                        out=vT[:], in0=qc, scalar=1.0 / H, in1=musq[:],
                        op0=OP.mult, op1=OP.subtract,
                    )
                    nc.vector.tensor_scalar_max(out=vT[:], in0=vT[:], scalar1=0.0)
                    lnv = iw.tile([128, GRP], f32, tag="lnv")
                    nc.scalar.activation(out=lnv[:], in_=vT[:], func=AF.Ln, bias=epsc[:])
                    qslice = qT[bt][:, g * GRP:(g + 1) * GRP]
                    nc.scalar.activation(out=qslice, in_=lnv[:], func=AF.Exp, scale=-0.5)
                    gamT = iw.tile([128, GRP], bf16, tag="gamT")
                    nc.vector.tensor_tensor(
                        out=gamT[:], in0=muT[:], in1=qslice, op=OP.mult)
                    nc.vector.reduce_sum(
                        out=gparts[bt][:, g:g + 1], in_=gamT[:],
                        axis=mybir.AxisListType.X)

            def emit_pass2(g):
                zt = get_zgrp(g)
                for bt in range(NBT):
                    yreg = stq[bt][:, 2 * F:2 * F + GRP * C]
                    for jg in range(GRP):
                        f = g * GRP + jg
                        nc.tensor.matmul(
                            out=yreg[:, jg * C:(jg + 1) * C],
                            lhsT=zt[:, jg, bt * 128:(bt + 1) * 128],
                            rhs=w2p_sb[:], start=True, stop=True,
                        )
                    ysc = iw.tile([128, GRP, C], bf16, tag="ysc")
                    yr3 = yreg.rearrange("p (j c) -> p j c", c=C)
                    qb = qT[bt][:, g * GRP:(g + 1) * GRP].unsqueeze(2).broadcast_to(
                        [128, GRP, C])
                    nc.vector.tensor_tensor(out=ysc[:], in0=yr3, in1=qb, op=OP.mult)
                    # accumulate scaled group into yacc on Pool (SBUF-only)
                    nc.gpsimd.tensor_tensor(
                        out=yacc[bt][:], in0=yacc[bt][:],
                        in1=ysc[:].rearrange("p j c -> p (j c)"), op=OP.add,
                    )

            # ---- emission: gathers prefetch 2 ahead; stats lag one call ----
            emit_gather(0)
            emit_gather(1)
            pending_stats = []
            grp_done = [False] * NGRP

            for c, (f0, nf) in enumerate(CALLS):
                gt = gouts[c]
                ntl = nf + (nf + 3) // 4
                # fc1 matmuls for this call
                zp_tiles = {}
                for jf in range(0, nf, 2):
                    zp = zps.tile([128, 2, BC], f32, tag="zp")
                    zp_tiles[jf] = zp
                    for k in range(2):
                        f = f0 + jf + k
                        a = (jf + k) % 4
                        tailt = nf + (jf + k) // 4
                        nc.tensor.matmul(
                            out=zp[:, k, :], lhsT=gpack_at(f, 0),
                            rhs=gt[:, jf + k, :], start=True, stop=False,
                        )
                        nc.tensor.matmul(
                            out=zp[:, k, :],
                            lhsT=gpack_at(f, 1)[32 * a:32 * a + 32, :],
                            rhs=gt[32 * a:32 * a + 32, tailt, :],
                            start=False, stop=True,
                            tile_position=(32 * a, 0),
                        )
                # prefetch next gather while fc1 runs
                if c + 2 < len(CALLS):
                    emit_gather(c + 2)
                # stats for the previous call (PE stays busy on fresh fc1)
                for (zsl, zqsl, bts) in pending_stats:
                    for bt in range(NBT):
                        nc.tensor.matmul(
                            out=bts[0][bt], lhsT=zsl[bt], rhs=onecol[:],
                            start=True, stop=True,
                        )
                        nc.tensor.matmul(
                            out=bts[1][bt], lhsT=zqsl[bt], rhs=onecol[:],
                            start=True, stop=True,
                        )
                pending_stats = []
                # relu + zsq per pair
                for jf in range(0, nf, 2):
                    zp = zp_tiles[jf]
                    f = f0 + jf
                    g0, g1 = f // GRP, (f + 1) // GRP
                    zt0 = get_zgrp(g0)
                    if g1 == g0:
                        zdst = zt0[:, f % GRP:f % GRP + 2, :]
                        if (jf // 2) % 5 < 3:
                            nc.scalar.activation(
                                out=zdst, in_=zp[:], func=AF.Relu)
                        else:
                            nc.vector.tensor_scalar_max(
                                out=zdst, in0=zp[:], scalar1=0.0)
                        zsq = zsqp.tile([128, 2, BC], bf16, tag="zsq")
                        nc.vector.tensor_tensor(
                            out=zsq[:], in0=zdst, in1=zdst, op=OP.mult)
                        for k in range(2):
                            ff = f + k
                            zsl = [zt0[:, ff % GRP, bt * 128:(bt + 1) * 128]
                                   for bt in range(NBT)]
                            zqsl = [zsq[:, k, bt * 128:(bt + 1) * 128]
                                    for bt in range(NBT)]
                            bts = ([stq[bt][:, ff:ff + 1] for bt in range(NBT)],
                                   [stq[bt][:, F + ff:F + ff + 1] for bt in range(NBT)])
                            pending_stats.append((zsl, zqsl,
                                                  (bts[0], bts[1])))
                    else:
                        # pair straddles a group boundary: handle singly
                        zt1 = get_zgrp(g1)
                        for k, zt in enumerate([zt0, zt1]):
                            ff = f + k
                            zdst = zt[:, ff % GRP, :]
                            if k == 0:
                                nc.scalar.activation(
                                    out=zdst, in_=zp[:, k, :], func=AF.Relu)
                            else:
                                nc.vector.tensor_scalar_max(
                                    out=zdst, in0=zp[:, k, :], scalar1=0.0)
                            zsq = zsqp.tile([128, 2, BC], bf16, tag="zsq")
                            nc.vector.tensor_tensor(
                                out=zsq[:, 0, :], in0=zdst,
                                in1=zdst, op=OP.mult)
                            zsl = [zt[:, ff % GRP, bt * 128:(bt + 1) * 128]
                                   for bt in range(NBT)]
                            zqsl = [zsq[:, 0, bt * 128:(bt + 1) * 128]
                                    for bt in range(NBT)]
                            pending_stats.append(
                                (zsl, zqsl,
                                 ([stq[bt][:, ff:ff + 1] for bt in range(NBT)],
                                  [stq[bt][:, F + ff:F + ff + 1] for bt in range(NBT)])))
                # group completion: interlude + pass2 once a group's stats
                # will all be emitted (they lag one call, so check c-1 range)
                fmax_statted = f0 - 1  # stats emitted so far cover < f0
                for g in range(NGRP):
                    if not grp_done[g] and (g + 1) * GRP - 1 <= fmax_statted:
                        grp_done[g] = True
                        emit_interlude(g)
                        emit_pass2(g)

            # flush remaining stats
            for (zsl, zqsl, bts) in pending_stats:
                for bt in range(NBT):
                    nc.tensor.matmul(
                        out=bts[0][bt], lhsT=zsl[bt], rhs=onecol[:],
                        start=True, stop=True,
                    )
                    nc.tensor.matmul(
                        out=bts[1][bt], lhsT=zqsl[bt], rhs=onecol[:],
                        start=True, stop=True,
                    )
            pending_stats = []
            for g in range(NGRP):
                if not grp_done[g]:
                    grp_done[g] = True
                    emit_interlude(g)
                    emit_pass2(g)

            # ---- final: tree-sum yacc (GRP*C cols = 20 forest-chunks of C),
            # rank-1 LN2 mean correction, scale by 1/F, store batch-major.
            for bt in range(NBT):
                ya = yacc[bt][:]
                u1 = iw.tile([128, 10 * C], bf16, tag="u1")
                nc.vector.tensor_tensor(
                    out=u1[:], in0=ya[:, 0:10 * C], in1=ya[:, 10 * C:20 * C], op=OP.add)
                u2 = iw.tile([128, 5 * C], bf16, tag="u2")
                nc.vector.tensor_tensor(
                    out=u2[:], in0=u1[:, 0:5 * C], in1=u1[:, 5 * C:10 * C], op=OP.add)
                u3 = iw.tile([128, 2 * C], bf16, tag="u3")
                nc.vector.tensor_tensor(
                    out=u3[:], in0=u2[:, 0:2 * C], in1=u2[:, 2 * C:4 * C], op=OP.add)
                u4 = iw.tile([128, C], f32, tag="u4")
                nc.vector.tensor_tensor(
                    out=u4[:], in0=u3[:, 0:C], in1=u3[:, C:2 * C], op=OP.add)
                u5 = iw.tile([128, C], f32, tag="u5")
                nc.vector.tensor_tensor(
                    out=u5[:], in0=u4[:], in1=u2[:, 4 * C:5 * C], op=OP.add)
                # gamsum over groups
                gtot = iw.tile([128, 1], f32, tag="gtot")
                nc.vector.reduce_sum(
                    out=gtot[:], in_=gparts[bt][:], axis=mybir.AxisListType.X)
                corr = iw.tile([128, C], f32, tag="corr")
                nc.vector.tensor_scalar(
                    out=corr[:], in0=cs2f_sb[:], scalar1=gtot[:],
                    scalar2=None, op0=OP.mult,
                )
                yfin = iw.tile([128, C], f32, tag="yfin")
                nc.vector.scalar_tensor_tensor(
                    out=yfin[:], in0=u5[:], scalar=1.0 / F, in1=corr[:],
                    op0=OP.mult, op1=OP.subtract,
                )
                nc.sync.dma_start(
                    out=y_d[bt * 128:(bt + 1) * 128, :], in_=yfin[:]
                )

    nc.compile()
    return nc


_CACHED = {}


def _get_program():
    if "nc" not in _CACHED:
        _CACHED["nc"] = _build_program()
    return _CACHED["nc"]


_LAST_RESULTS = None


def kernel(**inputs):
    global _LAST_RESULTS
    dev, x = _host_prep(inputs)
    nc = _get_program()

    in_maps = []
    for cid in range(NCORES):
        m = dict(dev)
        m["x_shard"] = np.ascontiguousarray(x[cid * BC:(cid + 1) * BC])
        in_maps.append(m)

    res = bass_utils.run_bass_kernel_spmd(nc, in_maps, core_ids=list(range(NCORES)))
    _LAST_RESULTS = res
    y = np.concatenate([r["y_out"] for r in res.results], axis=0)
    return y.astype(np.float32)


if __name__ == "__main__":
    # CoreSim smoke test on one core
    sys.path.insert(0, "/root/problem")
    import jax
    import reference

    with jax.default_device(jax.devices("cpu")[0]):
        inputs = {k: np.asarray(v) for k, v in reference.setup_inputs().items()}
    dev, x = _host_prep(inputs)
    nc = _build_program()
    from concourse.bass_interp import CoreSim

    sim = CoreSim(nc, trace=False)
    for k, v in dev.items():
        sim.tensor(k)[:] = v
    sim.tensor("x_shard")[:] = x[:BC]
    sim.simulate(check_with_hw=False)
    y0 = np.array(sim.tensor("y_out"))
    with jax.default_device(jax.devices("cpu")[0]):
        exp = np.asarray(reference.reference(**inputs))[:BC]
    err = np.abs(y0 - exp).max()
    rel2 = np.linalg.norm(y0 - exp) / (np.linalg.norm(exp) + 1e-30)
    print("sim maxabs:", err, " rel-l2:", rel2)
